# revision 48
# baseline (speedup 1.0000x reference)
"""Trainium2 Bass kernel for nn_NetStackedHourglass_2 keypoint reduction.

Full inputs in, full output out. Internally: pure data-parallel across 8
NeuronCores (32 batches each). Each core flattens its (batch, channel) pairs
to 640 rows = 5 groups of 128 partitions and computes per-(batch,channel)
masked spatial reductions.

Memory-regime optimizations vs the f32 baseline (measured on HW via
repeat-slope timing; baseline ~242 us/iter, this kernel ~121 us/iter):
  - all inputs are staged to bf16 on the host: HBM traffic halves
    (73.4 MB -> 36.7 MB per core). De-interleaved x/y vector planes keep
    every operand packed (stride-1), which the DVE 2x_1p fast mode needs.
  - all 7 input planes are packed into ONE DRAM tensor so each chunk is
    a single 3.5 MB DMA (7x fewer DMA instructions; ~108 us DMA floor).
  - compute is split across DVE and Act to hit the measured engine rates
    (DVE: accum ops always run 1x ~1.04 ns/elem, plain tensor-tensor 2x
    ~0.55; Act reduce ~1.1-1.4): DVE does the shared products and two
    fused stt vote reductions; Act reduces msum + four product planes.
    GpSimd/Pool is kept out of the dataflow - its software-engine latency
    stalls the pipeline whenever it feeds DVE or Act (measured 2x).

The tiny [B,20,*] -> [B,21,2] keypoint assembly (incl. the x64 vote
scale and per-chunk locx offsets) runs on host off the raw fp32
accumulators.
"""

import sys

if "/opt/trn_rl_repo" not in sys.path:
    sys.path.insert(0, "/opt/trn_rl_repo")

import numpy as np
from ml_dtypes import bfloat16, float8_e4m3

import concourse.bass as bass
import concourse.tile as tile
from concourse import mybir
from concourse.bass_utils import run_bass_kernel_spmd

N_CORES = 8
B_FULL = 256
B_SHARD = B_FULL // N_CORES  # 32
C = 20
RES = 64
SPATIAL = RES * RES          # 4096
ROWS = B_SHARD * C           # 640 (b,c) rows per core
P = 128                      # partitions
GROUPS = ROWS // P           # 5
CHUNK = 2048                 # spatial elements per tile
NCHUNK = SPATIAL // CHUNK    # 2
EPS = 1e-6

F32 = mybir.dt.float32
BF16 = mybir.dt.bfloat16


DEFAULT_VARIANT = "full"


def _build_program(repeat: int = 1, variant: str | None = None) -> bass.Bass:
    if variant is None:
        variant = DEFAULT_VARIANT
    nc = bass.Bass()

    # all 7 input planes packed host-side: [plane, row, spatial] with
    # planes 0:m 1:fd 2:bd 3:fx 4:fy 5:bx 6:by
    fp8_dma = variant in ("dma8", "full8")
    packed = nc.declare_dram_parameter(
        "packed",
        [7, ROWS, SPATIAL],
        mybir.dt.float8e4 if fp8_dma else BF16,
        isOutput=False,
    )
    # rows: locx for chunk 0 (host corrects later chunks via msum), then locy
    loc = nc.declare_dram_parameter("loc_const", [2, CHUNK], BF16, isOutput=False)
    # raw per-chunk accumulators: [row, chunk*8]; quantities:
    # 0:fx 1:fy 2:bx 3:by 4:lx 5:ly 6:msum 7:pad
    stats = nc.declare_dram_parameter("stats", [ROWS, NCHUNK * 8], F32, isOutput=True)

    MULT = mybir.AluOpType.mult

    with tile.TileContext(nc) as tc:
        with (
            tc.tile_pool(name="singles", bufs=1) as singles,
            tc.tile_pool(name="io", bufs=4) as io,
            tc.tile_pool(name="work", bufs=2) as work,
            tc.tile_pool(name="acc", bufs=4) as accp,
            tc.tile_pool(name="psum", bufs=2, space="PSUM") as psum,
        ):
            # --- constants: locx (chunk-0 pattern), locy ---
            locx = singles.tile([P, CHUNK], BF16, tag="locx")
            nc.gpsimd.dma_start(out=locx, in_=loc[0:1, :].to_broadcast([P, CHUNK]))
            locy = singles.tile([P, CHUNK], BF16, tag="locy")
            nc.gpsimd.dma_start(out=locy, in_=loc[1:2, :].to_broadcast([P, CHUNK]))

            def _group_body(g):
                r0 = g * P
                acc_t = None
                if variant not in ("dma", "dma8"):
                    acc_t = accp.tile([P, NCHUNK, 8], F32, tag="acc")
                for ci in range(NCHUNK):
                    c0 = ci * CHUNK
                    io_t = io.tile([P, 7, CHUNK], BF16, tag="io")

                    # two DMAs per chunk: planes 0-2 (m/fd/bd) land first so
                    # the first half of each engine's work can start before
                    # the vote planes (3-6) arrive
                    def plane_ap(a, b):
                        return packed[a:b, r0 : r0 + P, c0 : c0 + CHUNK].rearrange(
                            "t p c -> p t c"
                        )
                    if fp8_dma:
                        # SWDGE (gpsimd-initiated) DMA casts f8e4 -> bf16
                        nc.gpsimd.dma_start(out=io_t[:, 0:1, :], in_=plane_ap(0, 1))
                        nc.gpsimd.dma_start(out=io_t[:, 1:3, :], in_=plane_ap(1, 3))
                        nc.gpsimd.dma_start(out=io_t[:, 3:7, :], in_=plane_ap(3, 7))
                    else:
                        nc.sync.dma_start(out=io_t[:, 0:1, :], in_=plane_ap(0, 1))
                        nc.sync.dma_start(out=io_t[:, 1:3, :], in_=plane_ap(1, 3))
                        nc.sync.dma_start(out=io_t[:, 3:7, :], in_=plane_ap(3, 7))

                    if variant in ("dma", "dma8"):
                        continue

                    m_t = io_t[:, 0, :]
                    fd_t = io_t[:, 1, :]
                    bd_t = io_t[:, 2, :]
                    fx_t = io_t[:, 3, :]
                    fy_t = io_t[:, 4, :]
                    bx_t = io_t[:, 5, :]
                    by_t = io_t[:, 6, :]

                    t_f = work.tile([P, CHUNK], BF16, tag="t_f")
                    t_b = work.tile([P, CHUNK], BF16, tag="t_b")
                    p_bx = work.tile([P, CHUNK], BF16, tag="p_bx")
                    p_by = work.tile([P, CHUNK], BF16, tag="p_by")
                    p_lx = work.tile([P, CHUNK], BF16, tag="p_lx")
                    p_ly = work.tile([P, CHUNK], BF16, tag="p_ly")
                    pscr = psum.tile([P, CHUNK], F32, tag="pscr")

                    def act_reduce(in_t, qi):
                        # Act engine: out = copy(in), accum = sum(in)
                        nc.scalar.activation(
                            out=pscr,
                            in_=in_t,
                            func=mybir.ActivationFunctionType.Copy,
                            accum_out=acc_t[:, ci, qi : qi + 1],
                        )

                    def stt(out, in0, in1, qi):
                        # out = in0 * in1 ; accum = sum(out)  (DVE, 1x)
                        nc.vector.scalar_tensor_tensor(
                            out=out,
                            in0=in0,
                            scalar=1.0,
                            in1=in1,
                            op0=MULT,
                            op1=MULT,
                            accum_out=acc_t[:, ci, qi : qi + 1],
                        )

                    # Work split (all vote scales x64 fold into host assembly):
                    #   DVE:  products as plain tensor-tensor (2x mode) +
                    #         fused stt reductions for the fx/fy votes (1x)
                    #   Act:  msum + reductions of p_bx, p_by, p_lx, p_ly
                    #   Pool: nothing — its software-engine latency stalls
                    #         the pipeline whenever it feeds DVE or Act
                    # m-only ops first (just 0.5 MB landed) ...
                    act_reduce(m_t, 6)
                    nc.vector.tensor_mul(p_lx, m_t, locx)
                    nc.vector.tensor_mul(p_ly, m_t, locy)
                    act_reduce(p_lx, 4)
                    act_reduce(p_ly, 5)
                    # ... then ops needing fd/bd (second transfer)
                    nc.vector.tensor_mul(t_b, m_t, bd_t)
                    nc.vector.tensor_mul(t_f, m_t, fd_t)
                    # ... then ops needing the vote planes (3-6)
                    nc.vector.tensor_mul(p_bx, t_b, bx_t)
                    nc.vector.tensor_mul(p_by, t_b, by_t)
                    act_reduce(p_bx, 2)
                    act_reduce(p_by, 3)
                    stt(fd_t, t_f, fx_t, 0)
                    stt(fx_t, t_f, fy_t, 1)

                # raw accumulators to host; chunk-combine + normalize there
                if variant not in ("dma", "dma8"):
                    nc.sync.dma_start(
                        out=stats[r0 : r0 + P, :],
                        in_=acc_t.rearrange("p a b -> p (a b)"),
                    )

            def _groups_body():
                for g in range(GROUPS):
                    _group_body(g)

            if repeat == 1:
                _groups_body()
            else:
                # hardware loop: repeat the whole 5-group body `repeat`
                # times per dispatch (for repeat-slope timing) without
                # instruction-count growth. Two bodies per iteration
                # amortize the For_i all-engine barrier.
                assert repeat % 2 == 0, "timing repeat must be even"
                with tc.For_i(0, repeat // 2):
                    _groups_body()
                    _groups_body()

    from concourse.library_overlay import lower_extended_insts

    lower_extended_insts(nc)
    _legalize_waits(nc)
    return nc


def _legalize_waits(nc) -> None:
    """walrus codegen allows 1 sync-wait per instruction (2 for
    EventSemaphore). Hoist excess waits onto EventSemaphore carriers
    inserted just before the offending instruction on the same engine."""
    for f in nc.m.functions:
        for blk in f.blocks:
            insts = blk.instructions
            new_list = []
            changed = False
            for ins in insts:
                si = getattr(ins, "sync_info", None)
                ow = list(si.on_wait) if (si is not None and si.on_wait) else []
                cap = 2 if isinstance(ins, mybir.InstEventSemaphore) else 1
                if len(ow) > cap:
                    excess, keep = ow[:-cap], ow[-cap:]
                    for j in range(0, len(excess), 2):
                        ev = mybir.InstEventSemaphore(
                            name=f"{ins.name}-lw{j}", ins=[], outs=[]
                        )
                        ev.engine = ins.engine
                        ev.sync_info = mybir.SyncInfo(
                            on_wait=excess[j : j + 2], on_update=[]
                        )
                        new_list.append(ev)
                    ins.sync_info = mybir.SyncInfo(
                        on_wait=keep,
                        on_update=list(si.on_update) if si.on_update else [],
                    )
                    changed = True
                new_list.append(ins)
            if changed:
                blk.instructions.clear()
                blk.instructions.extend(new_list)


_PROGRAM_CACHE: dict = {}


def _get_program() -> bass.Bass:
    if "nc" not in _PROGRAM_CACHE:
        _PROGRAM_CACHE["nc"] = _build_program()
    return _PROGRAM_CACHE["nc"]


def _run_device(in_maps, trace=False, **kwargs):
    nc = _get_program()
    return run_bass_kernel_spmd(nc, in_maps, list(range(N_CORES)), trace=trace, **kwargs)


def _make_in_maps(front_vec, front_dis, back_vec, back_dis, ske_mask):
    fv = np.asarray(front_vec, dtype=np.float32).reshape(B_FULL, C, SPATIAL, 2)
    bv = np.asarray(back_vec, dtype=np.float32).reshape(B_FULL, C, SPATIAL, 2)
    # de-interleave x/y planes, downcast to bf16, flatten rows
    fvx = np.ascontiguousarray(fv[..., 0]).astype(bfloat16).reshape(-1, SPATIAL)
    fvy = np.ascontiguousarray(fv[..., 1]).astype(bfloat16).reshape(-1, SPATIAL)
    bvx = np.ascontiguousarray(bv[..., 0]).astype(bfloat16).reshape(-1, SPATIAL)
    bvy = np.ascontiguousarray(bv[..., 1]).astype(bfloat16).reshape(-1, SPATIAL)
    fd16 = np.asarray(front_dis, np.float32).astype(bfloat16).reshape(-1, SPATIAL)
    bd16 = np.asarray(back_dis, np.float32).astype(bfloat16).reshape(-1, SPATIAL)
    m16 = np.asarray(ske_mask, np.float32).astype(bfloat16).reshape(-1, SPATIAL)

    p = np.arange(CHUNK)
    loc_const = np.ascontiguousarray(
        np.stack([(p // RES), (p % RES)], axis=0).astype(bfloat16)
    )

    pack_dt = float8_e4m3 if DEFAULT_VARIANT.endswith("8") else bfloat16
    in_maps = []
    for i in range(N_CORES):
        sl = slice(i * ROWS, (i + 1) * ROWS)
        packed = np.ascontiguousarray(
            np.stack(
                [m16[sl], fd16[sl], bd16[sl], fvx[sl], fvy[sl], bvx[sl], bvy[sl]]
            ).astype(pack_dt)
        )
        in_maps.append({"packed": packed, "loc_const": loc_const})
    return in_maps


def _assemble(stats: np.ndarray) -> np.ndarray:
    """stats: [B, 20, NCHUNK*8] raw accumulators -> kp [B, 21, 2]."""
    B = stats.shape[0]
    acc = stats.reshape(B, C, NCHUNK, 8).astype(np.float32)
    s = acc.sum(axis=2)
    for ci in range(1, NCHUNK):
        # locx tile held the chunk-0 pattern; add the per-chunk offset
        s[:, :, 4] += np.float32(ci * (CHUNK // RES)) * acc[:, :, ci, 6]
    # vote scale (*res) is folded here rather than on-device
    s[:, :, 0:4] *= np.float32(RES)
    msum = s[:, :, 6]
    r = np.float32(1.0) / (msum + np.float32(EPS))
    F_ = np.stack([(s[:, :, 0] + s[:, :, 4]) * r, (s[:, :, 1] + s[:, :, 5]) * r], -1)
    Bk = np.stack([(s[:, :, 2] + s[:, :, 4]) * r, (s[:, :, 3] + s[:, :, 5]) * r], -1)

    root_terms = np.where(
        (msum[:, ::4] != 0.0)[..., None], Bk[:, ::4], np.float32(0.0)
    )  # [B,5,2]
    kp0 = root_terms.sum(axis=1, dtype=np.float32) / np.float32(5.0)  # [B,2]

    Fg = F_.reshape(B, 5, 4, 2)
    Bg = Bk.reshape(B, 5, 4, 2)
    tail = np.stack(
        [
            Fg[:, :, 3],
            (Fg[:, :, 2] + Bg[:, :, 3]) * np.float32(0.5),
            (Fg[:, :, 1] + Bg[:, :, 2]) * np.float32(0.5),
            (Fg[:, :, 0] + Bg[:, :, 1]) * np.float32(0.5),
        ],
        axis=2,
    )  # [B,5,4,2]
    kp = np.concatenate([kp0[:, None], tail.reshape(B, 20, 2)], axis=1)
    return (kp * np.float32(4.0)).astype(np.float32)


def kernel(front_vec, front_dis, back_vec, back_dis, ske_mask) -> np.ndarray:
    in_maps = _make_in_maps(front_vec, front_dis, back_vec, back_dis, ske_mask)
    res = _run_device(in_maps)
    stats = np.stack([np.asarray(res.results[i]["stats"]) for i in range(N_CORES)])
    stats = stats.reshape(B_FULL, C, NCHUNK * 8)
    return _assemble(stats)


# revision 49
# speedup vs baseline: 1.1016x; 1.1016x over previous
"""Trainium2 Bass kernel for nn_NetStackedHourglass_2 keypoint reduction.

Full inputs in, full output out. Internally: pure data-parallel across 8
NeuronCores (32 batches each). Each core flattens its (batch, channel) pairs
to 640 rows = 5 groups of 128 partitions and computes per-(batch,channel)
masked spatial reductions.

Memory-regime optimizations vs the f32 baseline (measured on HW via
repeat-slope timing; baseline ~242 us/iter, this kernel ~121 us/iter):
  - all inputs are staged to bf16 on the host: HBM traffic halves
    (73.4 MB -> 36.7 MB per core). De-interleaved x/y vector planes keep
    every operand packed (stride-1), which the DVE 2x_1p fast mode needs.
  - all 7 input planes are packed into ONE DRAM tensor so each chunk is
    a single 3.5 MB DMA (7x fewer DMA instructions; ~108 us DMA floor).
  - compute is split across DVE and Act to hit the measured engine rates
    (DVE: accum ops always run 1x ~1.04 ns/elem, plain tensor-tensor 2x
    ~0.55; Act reduce ~1.1-1.4): DVE does the shared products and two
    fused stt vote reductions; Act reduces msum + four product planes.
    GpSimd/Pool is kept out of the dataflow - its software-engine latency
    stalls the pipeline whenever it feeds DVE or Act (measured 2x).

The tiny [B,20,*] -> [B,21,2] keypoint assembly (incl. the x64 vote
scale and per-chunk locx offsets) runs on host off the raw fp32
accumulators.
"""

import sys

if "/opt/trn_rl_repo" not in sys.path:
    sys.path.insert(0, "/opt/trn_rl_repo")

import numpy as np
from ml_dtypes import bfloat16, float8_e4m3

import concourse.bass as bass
import concourse.tile as tile
from concourse import mybir
from concourse.bass_utils import run_bass_kernel_spmd

N_CORES = 8
B_FULL = 256
B_SHARD = B_FULL // N_CORES  # 32
C = 20
RES = 64
SPATIAL = RES * RES          # 4096
ROWS = B_SHARD * C           # 640 (b,c) rows per core
P = 128                      # partitions
GROUPS = ROWS // P           # 5
CHUNK = 2048                 # spatial elements per tile
NCHUNK = SPATIAL // CHUNK    # 2
EPS = 1e-6

F32 = mybir.dt.float32
BF16 = mybir.dt.bfloat16


DEFAULT_VARIANT = "full"


def _build_program(repeat: int = 1, variant: str | None = None) -> bass.Bass:
    if variant is None:
        variant = DEFAULT_VARIANT
    nc = bass.Bass()

    # all 7 input planes packed host-side: [plane, row, spatial] with
    # planes 0:m 1:fd 2:bd 3:fx 4:fy 5:bx 6:by
    fp8_dma = variant in ("dma8", "full8")
    packed = nc.declare_dram_parameter(
        "packed",
        [7, ROWS, SPATIAL],
        mybir.dt.float8e4 if fp8_dma else BF16,
        isOutput=False,
    )
    # rows: locx for chunk 0 (host corrects later chunks via msum), then locy
    loc = nc.declare_dram_parameter("loc_const", [2, CHUNK], BF16, isOutput=False)
    # raw per-chunk accumulators: [row, chunk*8]; quantities:
    # 0:fx 1:fy 2:bx 3:by 4:lx 5:ly 6:msum 7:pad
    stats = nc.declare_dram_parameter("stats", [ROWS, NCHUNK * 8], F32, isOutput=True)

    MULT = mybir.AluOpType.mult

    with tile.TileContext(nc) as tc:
        with (
            tc.tile_pool(name="singles", bufs=1) as singles,
            tc.tile_pool(name="io", bufs=4) as io,
            tc.tile_pool(name="work", bufs=2) as work,
            tc.tile_pool(name="acc", bufs=4) as accp,
            tc.tile_pool(name="psum", bufs=2, space="PSUM") as psum,
        ):
            # --- constants: locx (chunk-0 pattern), locy ---
            locx = singles.tile([P, CHUNK], BF16, tag="locx")
            nc.gpsimd.dma_start(out=locx, in_=loc[0:1, :].to_broadcast([P, CHUNK]))
            locy = singles.tile([P, CHUNK], BF16, tag="locy")
            nc.gpsimd.dma_start(out=locy, in_=loc[1:2, :].to_broadcast([P, CHUNK]))

            def _group_body(g):
                r0 = g * P
                acc_t = None
                if variant not in ("dma", "dma8"):
                    acc_t = accp.tile([P, NCHUNK, 8], F32, tag="acc")
                for ci in range(NCHUNK):
                    c0 = ci * CHUNK
                    io_t = io.tile([P, 7, CHUNK], BF16, tag="io")

                    # two DMAs per chunk: planes 0-2 (m/fd/bd) land first so
                    # the first half of each engine's work can start before
                    # the vote planes (3-6) arrive
                    def plane_ap(a, b):
                        return packed[a:b, r0 : r0 + P, c0 : c0 + CHUNK].rearrange(
                            "t p c -> p t c"
                        )
                    if fp8_dma:
                        # SWDGE (gpsimd-initiated) DMA casts f8e4 -> bf16
                        nc.gpsimd.dma_start(out=io_t[:, 0:3, :], in_=plane_ap(0, 3))
                        nc.gpsimd.dma_start(out=io_t[:, 3:7, :], in_=plane_ap(3, 7))
                    else:
                        nc.sync.dma_start(out=io_t[:, 0:3, :], in_=plane_ap(0, 3))
                        nc.sync.dma_start(out=io_t[:, 3:7, :], in_=plane_ap(3, 7))

                    if variant in ("dma", "dma8"):
                        continue

                    m_t = io_t[:, 0, :]
                    fd_t = io_t[:, 1, :]
                    bd_t = io_t[:, 2, :]
                    fx_t = io_t[:, 3, :]
                    fy_t = io_t[:, 4, :]
                    bx_t = io_t[:, 5, :]
                    by_t = io_t[:, 6, :]

                    t_f = work.tile([P, CHUNK], BF16, tag="t_f")
                    t_b = work.tile([P, CHUNK], BF16, tag="t_b")
                    p_bx = work.tile([P, CHUNK], BF16, tag="p_bx")
                    p_by = work.tile([P, CHUNK], BF16, tag="p_by")
                    p_lx = work.tile([P, CHUNK], BF16, tag="p_lx")
                    p_ly = work.tile([P, CHUNK], BF16, tag="p_ly")
                    pscr = psum.tile([P, CHUNK], F32, tag="pscr")

                    def act_reduce(in_t, qi):
                        # Act engine: out = copy(in), accum = sum(in)
                        nc.scalar.activation(
                            out=pscr,
                            in_=in_t,
                            func=mybir.ActivationFunctionType.Copy,
                            accum_out=acc_t[:, ci, qi : qi + 1],
                        )

                    def stt(out, in0, in1, qi):
                        # out = in0 * in1 ; accum = sum(out)  (DVE, 1x)
                        nc.vector.scalar_tensor_tensor(
                            out=out,
                            in0=in0,
                            scalar=1.0,
                            in1=in1,
                            op0=MULT,
                            op1=MULT,
                            accum_out=acc_t[:, ci, qi : qi + 1],
                        )

                    # Work split (all vote scales x64 fold into host assembly):
                    #   DVE:  products as plain tensor-tensor (2x mode) +
                    #         fused stt reductions for the fx/fy votes (1x)
                    #   Act:  msum + reductions of p_bx, p_by, p_lx, p_ly
                    #   Pool: nothing — its software-engine latency stalls
                    #         the pipeline whenever it feeds DVE or Act
                    # first-DMA-only ops up front (planes 0-2) ...
                    act_reduce(m_t, 6)
                    nc.vector.tensor_mul(t_b, m_t, bd_t)
                    nc.vector.tensor_mul(t_f, m_t, fd_t)
                    nc.vector.tensor_mul(p_lx, m_t, locx)
                    nc.vector.tensor_mul(p_ly, m_t, locy)
                    act_reduce(p_lx, 4)
                    act_reduce(p_ly, 5)
                    # ... then ops needing the vote planes (3-6)
                    nc.vector.tensor_mul(p_bx, t_b, bx_t)
                    nc.vector.tensor_mul(p_by, t_b, by_t)
                    act_reduce(p_bx, 2)
                    act_reduce(p_by, 3)
                    stt(fd_t, t_f, fx_t, 0)
                    stt(fx_t, t_f, fy_t, 1)

                # raw accumulators to host; chunk-combine + normalize there
                if variant not in ("dma", "dma8"):
                    nc.sync.dma_start(
                        out=stats[r0 : r0 + P, :],
                        in_=acc_t.rearrange("p a b -> p (a b)"),
                    )

            def _groups_body():
                for g in range(GROUPS):
                    _group_body(g)

            if repeat == 1:
                _groups_body()
            else:
                # hardware loop: repeat the whole 5-group body `repeat`
                # times per dispatch (for repeat-slope timing) without
                # instruction-count growth. Two bodies per iteration
                # amortize the For_i all-engine barrier.
                assert repeat % 2 == 0, "timing repeat must be even"
                with tc.For_i(0, repeat // 2):
                    _groups_body()
                    _groups_body()

    from concourse.library_overlay import lower_extended_insts

    lower_extended_insts(nc)
    _legalize_waits(nc)
    return nc


def _legalize_waits(nc) -> None:
    """walrus codegen allows 1 sync-wait per instruction (2 for
    EventSemaphore). Hoist excess waits onto EventSemaphore carriers
    inserted just before the offending instruction on the same engine."""
    for f in nc.m.functions:
        for blk in f.blocks:
            insts = blk.instructions
            new_list = []
            changed = False
            for ins in insts:
                si = getattr(ins, "sync_info", None)
                ow = list(si.on_wait) if (si is not None and si.on_wait) else []
                cap = 2 if isinstance(ins, mybir.InstEventSemaphore) else 1
                if len(ow) > cap:
                    excess, keep = ow[:-cap], ow[-cap:]
                    for j in range(0, len(excess), 2):
                        ev = mybir.InstEventSemaphore(
                            name=f"{ins.name}-lw{j}", ins=[], outs=[]
                        )
                        ev.engine = ins.engine
                        ev.sync_info = mybir.SyncInfo(
                            on_wait=excess[j : j + 2], on_update=[]
                        )
                        new_list.append(ev)
                    ins.sync_info = mybir.SyncInfo(
                        on_wait=keep,
                        on_update=list(si.on_update) if si.on_update else [],
                    )
                    changed = True
                new_list.append(ins)
            if changed:
                blk.instructions.clear()
                blk.instructions.extend(new_list)


_PROGRAM_CACHE: dict = {}


def _get_program() -> bass.Bass:
    if "nc" not in _PROGRAM_CACHE:
        _PROGRAM_CACHE["nc"] = _build_program()
    return _PROGRAM_CACHE["nc"]


def _run_device(in_maps, trace=False, **kwargs):
    nc = _get_program()
    return run_bass_kernel_spmd(nc, in_maps, list(range(N_CORES)), trace=trace, **kwargs)


def _make_in_maps(front_vec, front_dis, back_vec, back_dis, ske_mask):
    fv = np.asarray(front_vec, dtype=np.float32).reshape(B_FULL, C, SPATIAL, 2)
    bv = np.asarray(back_vec, dtype=np.float32).reshape(B_FULL, C, SPATIAL, 2)
    # de-interleave x/y planes, downcast to bf16, flatten rows
    fvx = np.ascontiguousarray(fv[..., 0]).astype(bfloat16).reshape(-1, SPATIAL)
    fvy = np.ascontiguousarray(fv[..., 1]).astype(bfloat16).reshape(-1, SPATIAL)
    bvx = np.ascontiguousarray(bv[..., 0]).astype(bfloat16).reshape(-1, SPATIAL)
    bvy = np.ascontiguousarray(bv[..., 1]).astype(bfloat16).reshape(-1, SPATIAL)
    fd16 = np.asarray(front_dis, np.float32).astype(bfloat16).reshape(-1, SPATIAL)
    bd16 = np.asarray(back_dis, np.float32).astype(bfloat16).reshape(-1, SPATIAL)
    m16 = np.asarray(ske_mask, np.float32).astype(bfloat16).reshape(-1, SPATIAL)

    p = np.arange(CHUNK)
    loc_const = np.ascontiguousarray(
        np.stack([(p // RES), (p % RES)], axis=0).astype(bfloat16)
    )

    pack_dt = float8_e4m3 if DEFAULT_VARIANT.endswith("8") else bfloat16
    in_maps = []
    for i in range(N_CORES):
        sl = slice(i * ROWS, (i + 1) * ROWS)
        packed = np.ascontiguousarray(
            np.stack(
                [m16[sl], fd16[sl], bd16[sl], fvx[sl], fvy[sl], bvx[sl], bvy[sl]]
            ).astype(pack_dt)
        )
        in_maps.append({"packed": packed, "loc_const": loc_const})
    return in_maps


def _assemble(stats: np.ndarray) -> np.ndarray:
    """stats: [B, 20, NCHUNK*8] raw accumulators -> kp [B, 21, 2]."""
    B = stats.shape[0]
    acc = stats.reshape(B, C, NCHUNK, 8).astype(np.float32)
    s = acc.sum(axis=2)
    for ci in range(1, NCHUNK):
        # locx tile held the chunk-0 pattern; add the per-chunk offset
        s[:, :, 4] += np.float32(ci * (CHUNK // RES)) * acc[:, :, ci, 6]
    # vote scale (*res) is folded here rather than on-device
    s[:, :, 0:4] *= np.float32(RES)
    msum = s[:, :, 6]
    r = np.float32(1.0) / (msum + np.float32(EPS))
    F_ = np.stack([(s[:, :, 0] + s[:, :, 4]) * r, (s[:, :, 1] + s[:, :, 5]) * r], -1)
    Bk = np.stack([(s[:, :, 2] + s[:, :, 4]) * r, (s[:, :, 3] + s[:, :, 5]) * r], -1)

    root_terms = np.where(
        (msum[:, ::4] != 0.0)[..., None], Bk[:, ::4], np.float32(0.0)
    )  # [B,5,2]
    kp0 = root_terms.sum(axis=1, dtype=np.float32) / np.float32(5.0)  # [B,2]

    Fg = F_.reshape(B, 5, 4, 2)
    Bg = Bk.reshape(B, 5, 4, 2)
    tail = np.stack(
        [
            Fg[:, :, 3],
            (Fg[:, :, 2] + Bg[:, :, 3]) * np.float32(0.5),
            (Fg[:, :, 1] + Bg[:, :, 2]) * np.float32(0.5),
            (Fg[:, :, 0] + Bg[:, :, 1]) * np.float32(0.5),
        ],
        axis=2,
    )  # [B,5,4,2]
    kp = np.concatenate([kp0[:, None], tail.reshape(B, 20, 2)], axis=1)
    return (kp * np.float32(4.0)).astype(np.float32)


def kernel(front_vec, front_dis, back_vec, back_dis, ske_mask) -> np.ndarray:
    in_maps = _make_in_maps(front_vec, front_dis, back_vec, back_dis, ske_mask)
    res = _run_device(in_maps)
    stats = np.stack([np.asarray(res.results[i]["stats"]) for i in range(N_CORES)])
    stats = stats.reshape(B_FULL, C, NCHUNK * 8)
    return _assemble(stats)
